# revision 29
# baseline (speedup 1.0000x reference)
"""Trainium2 Bass kernel for windowed local self-attention MLP.

Reference (per batch b, S=2048, D=H=256, A=16, W=33):
    h   = relu(x @ W1 + b1)
    logit[s,m] = (h Wq)[s].(h Wk)[s+A-m]/sqrt(H); attn = softmax(logit)
    att[s] = sum_m attn[s,m] (h Wv)[s+A-m]
    out = relu(att @ Wh + bh) @ Wo + bo

Sharding: data-parallel over batch, one element per core (B=8, 8 cores).

Algebraic folds (host, float64): M = Wq Wk^T (so K is h itself) and
Wv Wh (so the attention-apply emits pre-relu hid directly).  Softmax
normalization is deferred all the way to the output stage: hid' =
relu(sum_w e_w vs_w) is computed with UNnormalized masked exp weights,
and the final out matmul result is scaled by rec[t] = 1/den[t] per
token (exact for bh = 0; nonzero bh falls back to a host path).

Dataflow (all fp16 on chip except p1, PSUM fp32):
  p1: fp8 DoubleRow matmuls (x and W1 split hi=e4m3 / lo=e5m2 on host;
      3 DR terms, 0.5 cyc/row) -> ht [feat, token+A] fp16, zero padded.
  qt: M^T ht (fp16). vs: ht @ WvWh -> [token, feat] tiles, plus
      64-shifted overlap tiles built by SBUF->SBUF DMAs.
  attention in 64-token half-chunks j (window 96 = 64+2A), transposed
      orientation: logits^T [win, tok] via stationary ht-window;
      one exp per pair (ACT); band mask on Pool; denominator via tiny
      ones-matmuls on PE ([tok,1] psum at partition offset); apply with
      vs as stationary emits psa [feat, tok] directly -- no PE
      transposes and no transposed-weight drains at all.
  out: flipped orientation: stationary hid chunk, moving Wo [feat,2]
      -> [tok, 2] psum, drained with x rec[t] (the deferred softmax
      normalization), DMA'd as [S, 2].
"""
import sys

if "/opt/trn_rl_repo" not in sys.path:
    sys.path.insert(0, "/opt/trn_rl_repo")

import numpy as np
import ml_dtypes

import concourse.mybir as mybir
import concourse.tile as tile
from concourse import bacc
from concourse.bass_utils import run_bass_kernel_spmd

P = 128
S = 2048  # tokens per core
D = 256  # model dim
A = 16  # half window
NC = 16  # 128-token chunks
NH = 32  # 64-token half-chunks
WIN = 64 + 2 * A  # 96: per-half attention window
NCORES = 8

PADW = S + P + 2 * A  # padded token axis for ht, col = token + A (2208)
F32 = mybir.dt.float32
FP16 = mybir.dt.float16
E4 = mybir.dt.float8e4
E5 = mybir.dt.float8e5
DR = mybir.MatmulPerfMode.DoubleRow

import os
NDUMMY = int(os.environ.get("K_NDUMMY", "72"))  # PE warm-up matmuls
K_MASK = os.environ.get("K_MASK", "split")  # pool|dve|split
K_OLAG = int(os.environ.get("K_OLAG", "3"))
K_ALAG = int(os.environ.get("K_ALAG", "2"))
K_LSL = os.environ.get("K_LSL", "norm")
K_OPRI = os.environ.get("K_OPRI", "first")
K_ODMA = os.environ.get("K_ODMA", "2")  # out dma pieces: 2|3

_CACHED_NC = None
_LAST_RESULTS = None


def _build_nc():
    nc = bacc.Bacc(
        "TRN2",
        target_bir_lowering=False,
        debug=False,
        enable_asserts=False,
        num_devices=NCORES,
    )
    # blob: bias2 f32 [P,2] | hpk2 fp16 [P,261] (ones|bdT4|wo) | w1h e4
    # [P,2,256] | w1l e5 [P,2,256] | x8h[:,:,0:512] e4 | x8l[:,:,0:512] e5
    BLOB = 3604
    blob_d = nc.dram_tensor("blob", [P, BLOB], mybir.dt.uint8,
                            kind="ExternalInput").ap()
    x8h_d = nc.dram_tensor("x8h", [D, S - 512], E4, kind="ExternalInput").ap()
    x8l_d = nc.dram_tensor("x8l", [D, S - 512], E5, kind="ExternalInput").ap()
    wq_d = nc.dram_tensor("wq", [D, D], FP16, kind="ExternalInput").ap()
    wv_d = nc.dram_tensor("wv", [D, D], FP16, kind="ExternalInput").ap()
    # out_t[p, 2c:2c+2] = out for token 128c + p (host unscrambles)
    out_t = nc.dram_tensor("out_t", [P, 2 * NC], F32, kind="ExternalOutput").ap()

    Relu = mybir.ActivationFunctionType.Relu
    Exp = mybir.ActivationFunctionType.Exp
    Copy = mybir.ActivationFunctionType.Copy
    add_op = mybir.AluOpType.add
    max_op = mybir.AluOpType.max

    with tile.TileContext(nc) as tc:
        with (
            tc.tile_pool(name="persist", bufs=1) as persist,
            tc.tile_pool(name="work", bufs=8) as work,
            tc.tile_pool(name="psum", bufs=6, space="PSUM") as psum,
        ):
            # ---------------- persistent tiles ----------------
            blob = persist.tile([P, BLOB], mybir.dt.uint8)
            bias_sb = blob[:, 0:8].bitcast(F32)  # b1 lo | b1 hi
            hpk_sb = blob[:, 8:530].bitcast(FP16)
            ones_col = hpk_sb[:, 0:1]
            bdT4 = hpk_sb[:, 1:257].rearrange("p (a t) -> p a t", a=4)
            wo_sb = hpk_sb[:, 257:261]
            w1h_sb = blob[:, 530:1042].bitcast(E4).rearrange(
                "p (k m) -> p k m", k=2)
            w1l_sb = blob[:, 1042:1554].bitcast(E5).rearrange(
                "p (k m) -> p k m", k=2)
            x8h0 = blob[:, 1554:2578].bitcast(E4).rearrange(
                "p (k s) -> p k s", k=2)
            x8l0 = blob[:, 2578:3602].bitcast(E5).rearrange(
                "p (k s) -> p k s", k=2)
            wq_sb = persist.tile([P, 2, D], FP16)
            wv_sb = persist.tile([P, 2, D], FP16)

            x8h = persist.tile([P, 2, S - 512], E4)  # tokens 512..2048
            x8l = persist.tile([P, 2, S - 512], E5)
            ht = persist.tile([P, 2, PADW], FP16)  # col = token + A
            qt = persist.tile([P, 2, S], FP16)
            vs = persist.tile([P, NC + 1, D], FP16)  # row p tile t = tok 128t+p-A
            vs64o = persist.tile([P, NC, D], FP16)  # row p tile m = tok 128m+48+p
            hid = persist.tile([P, 2, S], FP16)
            rec_all = persist.tile([P, NC], F32)
            out_sb = persist.tile([P, NC, 2], F32)
            dmy = persist.tile([P, 64], FP16)

            # ---- startup: memsets, DMAs, PE warm-up ----
            nc.gpsimd.memset(dmy[:], 0.0)
            x8hr = x8h_d.rearrange("(k p) s -> p k s", p=P)
            x8lr = x8l_d.rearrange("(k p) s -> p k s", p=P)

            def rearr(w):
                return w.rearrange("(k p) h -> p k h", p=P)

            nc.sync.dma_start(blob[:], blob_d)
            nc.sync.dma_start(x8h[:], x8hr)
            nc.sync.dma_start(x8l[:], x8lr)
            nc.sync.dma_start(wq_sb[:], rearr(wq_d))
            nc.sync.dma_start(wv_sb[:], rearr(wv_d))
            for ko in range(2):
                nc.gpsimd.memset(ht[:, ko, 0:A], 0.0)
                nc.gpsimd.memset(ht[:, ko, S + A:PADW], 0.0)

            for _ in range(NDUMMY):
                psd = psum.tile([64, 64], F32, tag="bank", name="warm")
                nc.tensor.matmul(
                    psd[:], dmy[0:64, 0:64], dmy[0:64, 0:64],
                    start=True, stop=True,
                )

            # ---------------- dense phase bodies ----------------
            def p1_stripe(t):  # ht = relu(W1^T x + b1), fp8 DoubleRow
                if t == 0:
                    xh, xl = x8h0[:], x8l0[:]
                else:
                    sl = slice((t - 1) * 512, t * 512)
                    xh, xl = x8h[:, :, sl], x8l[:, :, sl]
                for hm in range(2):
                    ps = psum.tile([P, 512], F32, tag="bank")
                    wsl = slice(hm * P, (hm + 1) * P)
                    nc.tensor.matmul(
                        ps[:], w1h_sb[:, :, wsl], xh,
                        start=True, stop=False, perf_mode=DR,
                    )
                    nc.tensor.matmul(
                        ps[:], w1l_sb[:, :, wsl], xh,
                        start=False, stop=False, perf_mode=DR,
                    )
                    nc.tensor.matmul(
                        ps[:], w1h_sb[:, :, wsl], xl,
                        start=False, stop=True, perf_mode=DR,
                    )
                    dst = ht[:, hm, A + t * 512:A + (t + 1) * 512]
                    if hm == 0:
                        nc.scalar.activation(
                            dst, ps[:], Relu, bias=bias_sb[:, 0:1],
                        )
                    else:
                        nc.vector.tensor_scalar(
                            dst, ps[:], bias_sb[:, 1:2], 0.0, add_op, max_op,
                        )

            def p23_stripe(t):  # qt = M^T ht (M = Wq Wk^T, host-folded)
                for hm in range(2):
                    psq = psum.tile([P, 512], F32, tag="bank")
                    for k in range(2):
                        nc.tensor.matmul(
                            psq[:], wq_sb[:, k, hm * P:(hm + 1) * P],
                            ht[:, k, A + t * 512:A + (t + 1) * 512],
                            start=(k == 0), stop=(k == 1),
                        )
                    dst = qt[:, hm, t * 512:(t + 1) * 512]
                    if hm == 0:
                        nc.scalar.copy(dst, psq[:])
                    else:
                        nc.vector.tensor_copy(dst, psq[:])

            p4_alt = [0]

            def p4_group(v0, n):  # shifted V tiles + 64-shifted overlap DMAs
                psv = psum.tile([P, 2, D], F32, tag="bank")
                for i in range(n):
                    for k in range(2):
                        nc.tensor.matmul(
                            psv[:, i, :],
                            ht[:, k, (v0 + i) * P:(v0 + i + 1) * P],
                            wv_sb[:, k, :],
                            start=(k == 0), stop=(k == 1),
                        )
                if p4_alt[0] % 3 != 2:
                    nc.scalar.copy(vs[:, v0:v0 + n, :], psv[:, 0:n, :])
                else:
                    nc.vector.tensor_copy(vs[:, v0:v0 + n, :], psv[:, 0:n, :])
                p4_alt[0] += 1

            def shift_group(m0, m1):  # vs64o[m0:m1] from vs tiles m0..m1+1
                nc.sync.dma_start(
                    vs64o[0:64, m0:m1, :], vs[64:128, m0:m1, :]
                )
                nc.sync.dma_start(
                    vs64o[64:128, m0:m1, :], vs[0:64, m0 + 1:m1 + 1, :]
                )

            # ---------------- attention stage bodies ----------------
            pair_state = {}

            def p5_logits(cp):  # PE: transposed-window QK logits + ACT exp
                psl = psum.tile([96, 4, 64], F32, tag="bank", name="logit")
                for ci in range(2):
                    for h in range(2):
                        j = 4 * cp + 2 * ci + h
                        for k in range(2):
                            nc.tensor.matmul(
                                psl[:, 2 * ci + h, :],
                                ht[:, k, 64 * j:64 * j + WIN],
                                qt[:, k, 64 * j:64 * (j + 1)],
                                start=(k == 0), stop=(k == 1),
                            )
                e = work.tile([96, 4, 64], FP16, tag="e")
                nc.scalar.activation(e[:], psl[:], Exp, scale=0.0625)
                pair_state[cp] = e

            def p5_mask(cp):  # band mask (keeps weights unnormalized)
                e = pair_state.pop(cp)
                em = work.tile([96, 4, 64], FP16, tag="em")
                if K_MASK == "pool":
                    eng = nc.gpsimd
                elif K_MASK == "dve":
                    eng = nc.vector
                else:
                    eng = nc.vector if cp >= 6 else nc.gpsimd
                eng.tensor_mul(em[:], e[:], bdT4[0:96, :, :])
                pair_state[("em", cp)] = em

            def p5_apply(cp):  # PE: V-apply + den matmuls; drains + recip
                em = pair_state.pop(("em", cp))
                psa = psum.tile([P, 2, 4, 64], F32, tag="bank", name="attp")
                dp = psum.tile([P, 2], F32, tag="dband", name="denp", bufs=2)
                for ci in range(2):
                    for h in range(2):
                        j = 4 * cp + 2 * ci + h
                        if j % 2 == 0:
                            vsrc = vs[0:WIN, j // 2, :]
                        else:
                            vsrc = vs64o[0:WIN, (j - 1) // 2, :]
                        for fm in range(2):
                            nc.tensor.matmul(
                                psa[:, fm, 2 * ci + h, :],
                                vsrc[:, fm * P:(fm + 1) * P],
                                em[:, 2 * ci + h, :],
                                start=True, stop=True,
                            )
                        nc.tensor.matmul(
                            dp[64 * h:64 * (h + 1), ci:ci + 1],
                            em[:, 2 * ci + h, :],
                            ones_col[0:WIN, :],
                            start=True, stop=True,
                        )
                dst = hid[:, :, cp * 256:(cp + 1) * 256].rearrange(
                    "p k (a t) -> p k a t", a=4)
                if cp % 2 == 0:
                    nc.scalar.activation(dst, psa[:], Relu)
                else:
                    nc.vector.tensor_scalar(
                        dst, psa[:], 0.0, 0.0, add_op, max_op,
                    )
                nc.vector.reciprocal(rec_all[:, 2 * cp:2 * cp + 2], dp[:])

            def p7_pair(cp):  # flipped out matmuls + rec-scaled drains
                pso = psum.tile([P, 4], F32, tag="dband", name="outp", bufs=2)
                for ci in range(2):
                    c = 2 * cp + ci
                    for k in range(2):
                        nc.tensor.matmul(
                            pso[:, ci * 2:(ci + 1) * 2],
                            hid[:, k, c * P:(c + 1) * P],
                            wo_sb[:, 2 * k:2 * (k + 1)],
                            start=(k == 0), stop=(k == 1),
                        )
                for ci in range(2):
                    nc.vector.tensor_scalar_mul(
                        out_sb[:, 2 * cp + ci, :], pso[:, 2 * ci:2 * ci + 2],
                        rec_all[:, 2 * cp + ci:2 * cp + ci + 1],
                    )

            # ---------------- unified wave emission ----------------
            def stage(kind, i):
                if kind == "p1":
                    p1_stripe(i)
                elif kind == "qt":
                    p23_stripe(i)
                elif kind == "v":
                    p4_group(2 * i, 2) if i < 8 else p4_group(NC, 1)
                elif kind == "S":
                    shift_group(*i)
                elif kind == "L":
                    p5_logits(i)
                elif kind == "M":
                    p5_mask(i)
                elif kind == "A":
                    p5_apply(i)
                elif kind == "O":
                    p7_pair(i)

            P1_SLOTS = [0, 1, 2, 3]
            QT_SLOTS = [2, 3, 4, 5]
            V_SLOTS = [1, 2, 2, 3, 3, 4, 4, 5, 5]
            # shift groups (odd-tile ranges) placed right after their last
            # vs producer group
            S_EVENTS = [(3, (0, 5)), (4, (5, 9)), (5, (9, 13)), (6, (13, 16))]
            if K_LSL == "tight":
                L_SLOTS = [3, 3, 4, 4, 5, 5, 6, 6]
            else:
                L_SLOTS = [3, 4, 5, 5, 6, 6, 7, 7]
            M_LAG, A_LAG, O_LAG = 1, K_ALAG, K_OLAG
            opri = 0 if K_OPRI == "first" else 6
            ORDER = {"O": opri, "qt": 1, "p1": 2, "L": 3, "M": 4, "A": 5,
                     "v": 7, "S": 8}
            ev = []
            for t, sl in enumerate(P1_SLOTS):
                ev.append((sl, ORDER["p1"], "p1", t))
            for t, sl in enumerate(QT_SLOTS):
                ev.append((sl, ORDER["qt"], "qt", t))
            for j, sl in enumerate(V_SLOTS):
                ev.append((sl, ORDER["v"], "v", j))
            for sl, rng_ in S_EVENTS:
                ev.append((sl, ORDER["S"], "S", rng_))
            for k, sl in enumerate(L_SLOTS):
                ev.append((sl, ORDER["L"], "L", k))
                ev.append((sl + M_LAG, ORDER["M"], "M", k))
                ev.append((sl + A_LAG, ORDER["A"], "A", k))
                ev.append((sl + O_LAG, ORDER["O"], "O", k))
            nslots = max(s for s, _, _, _ in ev) + 1
            slots = [[] for _ in range(nslots)]
            for sl, pri, kind, idx in sorted(
                ev, key=lambda x: (x[0], x[1], str(x[3]))
            ):
                slots[sl].append((kind, idx))
            for m, slot in enumerate(slots):
                for kind, i in slot:
                    stage(kind, i)
                if K_ODMA == "3":
                    if m == 8:
                        nc.sync.dma_start(out_t[:, 0:16], out_sb[:, 0:8, :])
                    if m == 9:
                        nc.sync.dma_start(out_t[:, 16:24], out_sb[:, 8:12, :])
                else:
                    if m == 9:
                        nc.sync.dma_start(out_t[:, 0:24], out_sb[:, 0:12, :])
            nc.sync.dma_start(out_t[:, 24:32], out_sb[:, 12:16, :])

    nc.compile()
    return nc


def _get_nc():
    global _CACHED_NC
    if _CACHED_NC is None:
        _CACHED_NC = _build_nc()
    return _CACHED_NC


def _bandT4():
    # bandT[w, t] = 1 iff t <= w <= t + 2A  (window col = token offset + A)
    w = np.arange(WIN)[:, None]
    t = np.arange(64)[None, :]
    m = ((w >= t) & (w <= t + 2 * A)).astype(np.float16)
    out = np.zeros((P, 4 * 64), np.float16)
    out[:WIN] = np.tile(m, (1, 4))
    return out


def _host_fallback(x, W1, b1, Wq, Wk, Wv, Wh, bh, Wo, bo):
    h = np.maximum(x @ W1 + b1.reshape(1, 1, -1), 0.0)
    B = x.shape[0]
    pad = np.pad(h, ((0, 0), (A, A), (0, 0)))
    idx = np.arange(S)[:, None] + 2 * A - np.arange(2 * A + 1)[None, :]
    nei = pad[:, idx]
    q = h @ Wq
    kk = nei @ Wk
    vv = nei @ Wv
    lg = np.einsum("bsh,bswh->bsw", q, kk) / np.sqrt(float(D))
    lg -= lg.max(-1, keepdims=True)
    e = np.exp(lg)
    at = e / e.sum(-1, keepdims=True)
    att = np.einsum("bsw,bswh->bsh", at, vv)
    hid = np.maximum(att @ Wh + bh.reshape(1, 1, -1), 0.0)
    return (hid @ Wo + bo.reshape(1, 1, -1)).astype(np.float32)


def kernel(x, W1, b1, Wq, Wk, Wv, Wh, bh, Wo, bo, **_unused):
    x = np.asarray(x, dtype=np.float32)
    W1 = np.asarray(W1, dtype=np.float32)
    Wq = np.asarray(Wq, dtype=np.float32)
    Wk = np.asarray(Wk, dtype=np.float32)
    Wv = np.asarray(Wv, dtype=np.float32)
    Wh = np.asarray(Wh, dtype=np.float32)
    Wo = np.asarray(Wo, dtype=np.float32)
    b1f = np.asarray(b1, dtype=np.float32).reshape(D)
    bhf = np.asarray(bh, dtype=np.float32).reshape(D)
    bof = np.asarray(bo, dtype=np.float32).reshape(2)

    if np.any(bhf != 0.0):
        # deferred-normalization fold requires bh == 0; exact host path
        return _host_fallback(x, W1, np.asarray(b1, np.float32), Wq, Wk,
                              Wv, Wh, np.asarray(bh, np.float32), Wo,
                              np.asarray(bo, np.float32))

    wqm = (Wq.astype(np.float64) @ Wk.astype(np.float64).T).astype(np.float16)
    wvh = (Wv.astype(np.float64) @ Wh.astype(np.float64)).astype(np.float16)
    w1h = W1.astype(ml_dtypes.float8_e4m3)
    w1l = (W1 - w1h.astype(np.float32)).astype(ml_dtypes.float8_e5m2)
    # [P, 2, 256] device layout for the blob: [p, k, m] = W[k*128+p, m]
    w1h_r = np.ascontiguousarray(w1h.reshape(2, P, D).transpose(1, 0, 2))
    w1l_r = np.ascontiguousarray(w1l.reshape(2, P, D).transpose(1, 0, 2))

    bias2 = np.stack([b1f[:P], b1f[P:]], axis=1).astype(np.float32)
    wo_re = np.ascontiguousarray(
        Wo.reshape(2, P, 2).transpose(1, 0, 2).reshape(P, 4)
    ).astype(np.float16)
    ones_col = np.ones((P, 1), np.float16)
    hpk2 = np.concatenate([ones_col, _bandT4(), wo_re], axis=1).astype(
        np.float16)

    def by(a):
        return np.ascontiguousarray(a).view(np.uint8).reshape(P, -1)

    blob_const = np.concatenate(
        [by(bias2), by(hpk2), by(w1h_r), by(w1l_r)], axis=1)

    nc = _get_nc()
    in_maps = []
    for b in range(NCORES):
        xt = np.ascontiguousarray(x[b].T)
        x8h = xt.astype(ml_dtypes.float8_e4m3)
        x8l = (xt - x8h.astype(np.float32)).astype(ml_dtypes.float8_e5m2)
        x8h_r = x8h.reshape(2, P, S).transpose(1, 0, 2)  # [p, k, s]
        x8l_r = x8l.reshape(2, P, S).transpose(1, 0, 2)
        blob = np.concatenate(
            [blob_const, by(x8h_r[:, :, 0:512]), by(x8l_r[:, :, 0:512]),
             np.zeros((P, 2), np.uint8)], axis=1)
        in_maps.append({
            "blob": blob,
            "x8h": np.ascontiguousarray(x8h[:, 512:]),
            "x8l": np.ascontiguousarray(x8l[:, 512:]),
            "wq": wqm, "wv": wvh,
        })
    # one retry: the shared device occasionally throws a transient
    # NRT_EXEC_UNIT_UNRECOVERABLE; re-running recovers it
    try:
        res = run_bass_kernel_spmd(nc, in_maps, core_ids=list(range(NCORES)))
    except Exception:
        res = run_bass_kernel_spmd(nc, in_maps, core_ids=list(range(NCORES)))
    global _LAST_RESULTS
    _LAST_RESULTS = res
    out = np.stack(
        [
            res.results[b]["out_t"].reshape(P, NC, 2).transpose(1, 0, 2)
            .reshape(S, 2) + bof[None, :]
            for b in range(NCORES)
        ],
        axis=0,
    )
    return out.astype(np.float32)


if __name__ == "__main__":
    rng = np.random.default_rng(0)
    ins = {
        "x": rng.standard_normal((8, S, D), dtype=np.float32),
        "W1": (rng.standard_normal((D, D), dtype=np.float32) / 16),
        "b1": np.zeros((1, 1, D), np.float32),
        "Wq": (rng.standard_normal((D, D), dtype=np.float32) / 16),
        "Wk": (rng.standard_normal((D, D), dtype=np.float32) / 16),
        "Wv": (rng.standard_normal((D, D), dtype=np.float32) / 16),
        "Wh": (rng.standard_normal((D, D), dtype=np.float32) / 16),
        "bh": np.zeros((1, 1, D), np.float32),
        "Wo": (rng.standard_normal((D, 2), dtype=np.float32) / 16),
        "bo": np.zeros((1, 1, 2), np.float32),
    }
    y = kernel(**ins)
    print("kernel output", y.shape, y.dtype, float(np.abs(y).max()))
